# revision 12
# baseline (speedup 1.0000x reference)
"""Trainium2 Bass kernel for nn_ContinualSVGP (sparse-GP posterior prediction).

Math (per hyper h, output o; M=64 inducing, D=8, N=32768 points):
    kfu[n,m] = var * exp(-0.5*||x_n/ls - z_m/ls||^2)
    pred_mu  = kfu @ w            where w = Linv^T (Linv u_mean),  Linv = chol(kuu)^-1
    pred_var = var + diag(kfu (Q2-Q1) kfu^T),  Q1 = Kuu^-1, Q2 = C^T C,
               C = (u_tril / diag(L)) ^T Linv  (faithful to the reference's
               upper-triangular-solve-of-a-lower-matrix quirk).

Device mapping (per core, N sharded 8 ways -> N_loc=4096, blk=512):
    mm1 (bf16 3-term split, K=102, ho-pair block-diag):  s = xaug^T W_aug
        s[m,n] = x_n.z_m/ls^2 - 0.5||x_n/ls||^2 + (log var - 0.5||z_m/ls||^2)
    exp (ACT -> f32r):  kfu = exp(s)                       [128=2ho x 512]
    mm2 (f32r):         t = blockdiag(Q,Q') kfu            [128 x 512]
    prod (DVE -> f32r): g = kfu * t
    mm3a/b (f32r, M=32 overlap-accumulate into one bank):
        rows 2p+s     += ones . g      (pred_var - var)
        rows 16+2p+s  += w . kfu       (pred_mu)
    mmv (bf16 K=2): pre-writes the bank with [var (rows<16); 0] via var_hi+var_lo
    ACT copies the [32,512] bank into SBUF staging; one DMA out per core.
"""

import numpy as np
import ml_dtypes

H, O, M, D = 4, 4, 64, 8
N = 32768
JITTER = 1e-4
NCORES = 8
N_LOC = N // NCORES
BLK = 512
NBLK = N_LOC // BLK
NHO = H * O          # 16
NPAIR = NHO // 2     # 8
KSPLIT = 3 * (D + D + 1)   # 51 rows per ho after 3-term bf16 split
BF16 = ml_dtypes.bfloat16

_cache = {}


def _rne11(a):
    """Round float32 array to f32r precision (RNE to 11 mantissa bits)."""
    b = np.asarray(a, np.float32).view(np.uint32)
    shift = 23 - 11
    add = np.uint32((1 << (shift - 1)) - 1)
    r = (((b + add + ((b >> np.uint32(shift)) & np.uint32(1))) >> np.uint32(shift))
         << np.uint32(shift))
    return r.view(np.float32)


def _bf16_split(v):
    """v (f64) -> (hi, lo) bf16 pair with hi+lo ~ v to ~2^-17."""
    hi = np.asarray(v, np.float64).astype(BF16)
    lo = (np.asarray(v, np.float64) - hi.astype(np.float64)).astype(BF16)
    return hi, lo


def _fwd_sub_inv(L):
    """Inverse of a lower-triangular matrix via forward substitution (f64)."""
    m = L.shape[0]
    inv = np.zeros_like(L)
    for i in range(m):
        inv[i, i] = 1.0 / L[i, i]
        for j in range(i):
            inv[i, j] = -np.dot(L[i, j:i], inv[j:i, j]) / L[i, i]
    return inv


def _host_precompute(x, z, u_mean, u_tril_vec, log_ls, log_var):
    """Build all device constants. Everything f64 internally."""
    x = x.astype(np.float64)
    z = z.astype(np.float64)
    um = u_mean.astype(np.float64)
    utv = u_tril_vec.astype(np.float64)
    lls = log_ls.astype(np.float64)
    lv = log_var.astype(np.float64)

    # xaug rows (f64): 0:8 x_d, 8:16 x_d^2, 16 ones  -> [17, N]
    xr = np.empty((2 * D + 1, N), np.float64)
    xr[0:D] = x.T
    xr[D:2 * D] = (x.T) ** 2
    xr[2 * D] = 1.0
    x_hi, x_lo = _bf16_split(xr)
    xaug = np.empty((2 * KSPLIT, N), BF16)
    xaug[0:17] = x_hi
    xaug[17:34] = x_hi
    xaug[34:51] = x_lo
    xaug[51:102] = xaug[0:51]

    tril_i, tril_j = np.tril_indices(M)
    mm1w = np.zeros((2 * KSPLIT, NPAIR * 128), BF16)
    mm2w = np.zeros((128, NPAIR * 128), np.float32)
    mm3aw = np.zeros((128, NPAIR * 32), np.float32)
    mm3bw = np.zeros((128, NPAIR * 32), np.float32)
    mmvw = np.zeros((2, 32), BF16)

    for ho in range(NHO):
        h, o = divmod(ho, O)
        p, s = divmod(ho, 2)
        ls = np.exp(lls[h, o])
        var = np.exp(lv[h, o])
        il2 = ls ** -2
        zs = z[o] / ls
        zn = (zs ** 2).sum(1)
        kuu = var * np.exp(-0.5 * (zn[:, None] + zn[None, :] - 2.0 * zs @ zs.T)) \
            + JITTER * np.eye(M)
        L = np.linalg.cholesky(kuu)
        Linv = _fwd_sub_inv(L)
        ut = np.zeros((M, M))
        ut[tril_i, tril_j] = utv[o]
        C = (ut / np.diag(L)[:, None]).T @ Linv
        Q = C.T @ C - Linv.T @ Linv          # symmetric [M, M]
        w = Linv.T @ (Linv @ um[o][:, 0])    # [M]

        # rhs_aug [17, M] (mm1 stationary weights, pre-split)
        ra = np.empty((2 * D + 1, M), np.float64)
        ra[0:D] = (z[o] * il2[None, :]).T
        ra[D:2 * D] = np.repeat((-0.5 * il2)[:, None], M, axis=1)
        ra[2 * D] = lv[h, o] - 0.5 * zn
        w_hi, w_lo = _bf16_split(ra)
        col0 = 64 * s
        mm1w[51 * s:51 * s + 17, 128 * p + col0:128 * p + col0 + 64] = w_hi
        mm1w[51 * s + 17:51 * s + 34, 128 * p + col0:128 * p + col0 + 64] = w_lo
        mm1w[51 * s + 34:51 * s + 51, 128 * p + col0:128 * p + col0 + 64] = w_hi

        mm2w[64 * s:64 * s + 64, 128 * p + col0:128 * p + col0 + 64] = \
            Q.astype(np.float32)
        mm3aw[64 * s:64 * s + 64, 32 * p + 2 * p + s] = 1.0
        mm3bw[64 * s:64 * s + 64, 32 * p + 16 + 2 * p + s] = \
            w.astype(np.float32)
        vh = np.float64(np.array(var, np.float64).astype(BF16))
        mmvw[0, ho] = np.float32(vh)
        mmvw[1, ho] = np.float32(var - vh)

    cR = np.concatenate([mm2w, mm3aw, mm3bw], axis=1)   # [128, 1536]
    cR = _rne11(cR)
    return xaug, mm1w, cR, mmvw


def _build_program():
    import concourse.bass as bass
    import concourse.mybir as mybir
    from concourse.tile import TileContext
    from concourse.tile_rust import add_dep_helper

    BF = mybir.dt.bfloat16
    FR = mybir.dt.float32r
    F32 = mybir.dt.float32

    nc = bass.Bass("TRN2", target_bir_lowering=False, debug=False,
                   num_devices=NCORES)
    xaug_ext = nc.dram_tensor("xaug", [2 * KSPLIT, N_LOC], BF,
                              kind="ExternalInput")
    mm1w_ext = nc.dram_tensor("mm1w", [2 * KSPLIT, NPAIR * 128], BF,
                              kind="ExternalInput")
    cr_ext = nc.dram_tensor("cR", [128, 1536], FR, kind="ExternalInput")
    mmvw_ext = nc.dram_tensor("mmvw", [2, 32], BF, kind="ExternalInput")
    out_ext = nc.dram_tensor("outp", [32, N_LOC], F32, kind="ExternalOutput")

    with TileContext(nc) as tc:
        with tc.tile_pool(name="sb", bufs=1) as sb, \
             tc.tile_pool(name="kp", bufs=8) as kp, \
             tc.tile_pool(name="gp", bufs=8) as gp, \
             tc.tile_pool(name="ps", bufs=3, space="PSUM") as ps, \
             tc.tile_pool(name="po", bufs=2, space="PSUM") as po:
            funnel = []
            xaug_d = sb.tile([2 * KSPLIT, N_LOC], BF, tag="xaug_d")
            funnel.append(nc.sync.dma_start(out=xaug_d[:], in_=xaug_ext[:]).ins)
            mm1w_d = sb.tile([2 * KSPLIT, NPAIR * 128], BF, tag="mm1w_d")
            funnel.append(nc.sync.dma_start(out=mm1w_d[:], in_=mm1w_ext[:]).ins)
            cr_d = sb.tile([128, 1536], FR, tag="cr_d")
            funnel.append(nc.sync.dma_start(out=cr_d[:], in_=cr_ext[:]).ins)
            mmvw_d = sb.tile([2, 32], BF, tag="mmvw_d")
            funnel.append(nc.sync.dma_start(out=mmvw_d[:], in_=mmvw_ext[:]).ins)

            # launder DMA'd inputs through one engine copy each (DMA-queue
            # sem waits are not elidable; engine sems are)
            xaug = sb.tile([2 * KSPLIT, N_LOC], BF, tag="xaug")
            nc.scalar.copy(xaug[:], xaug_d[:])
            mm1w = sb.tile([2 * KSPLIT, NPAIR * 128], BF, tag="mm1w")
            nc.scalar.copy(mm1w[:], mm1w_d[:])
            cr = sb.tile([128, 1536], FR, tag="cr")
            nc.vector.tensor_copy(cr[:], cr_d[:])
            mmvw = sb.tile([2, 32], BF, tag="mmvw")
            nc.vector.tensor_copy(mmvw[:], mmvw_d[:])
            onesrow = sb.tile([2, BLK], BF, tag="onesrow")
            nc.vector.memset(onesrow[:], 1.0)
            dummy_bf = sb.tile([1, 1], BF, tag="dummy_bf")
            nc.vector.memset(dummy_bf[:], 0.0)
            # ACT-written scratch: absorber reads merge with same-sem deps
            dummy_srcA = sb.tile([1, 1], mybir.dt.float32, tag="dummy_srcA")
            nc.scalar.copy(dummy_srcA[:], dummy_bf[:])

            staging = sb.tile([32, N_LOC], mybir.dt.float32, tag="staging")

            prod_hist = []   # (b, p) -> prod instruction, for WAR absorbers
            exp_hist = []
            last_act = None
            last_pe = None
            last_dve = None

            for b in range(NBLK):
                ps_o = po.tile([32, BLK], mybir.dt.float32, tag="ps_o")
                mmv = nc.tensor.matmul(ps_o[:], mmvw[:], onesrow[:],
                                       start=True, stop=False)
                last_pe = mmv.ins
                blk_pre = []
                if b > 0:
                    # per-blk absorbers: this blk's exps/prods reuse slots
                    # whose last writers/readers are blk b-1's ops; one
                    # observation of the final tick covers the whole blk.
                    prev_prod = prod_hist[b * NPAIR - 1]
                    prev_exp = exp_hist[b * NPAIR - 1]
                    t1 = sb.tile([1, 1], mybir.dt.float32, tag=f"aab1_{b}")
                    aab1 = nc.scalar.copy(t1[:], dummy_bf[:])
                    add_dep_helper(aab1.ins, prev_prod, True, "ACT sees DVE")
                    t2 = sb.tile([1, 1], mybir.dt.float32, tag=f"aab2_{b}")
                    aab2 = nc.scalar.copy(t2[:], dummy_srcA[:])
                    add_dep_helper(aab2.ins, prev_exp, True, "ACT WAW")
                    t3 = sb.tile([1, 1], mybir.dt.float32, tag=f"dvb_{b}")
                    dvb = nc.vector.memset(t3[:], 0.0)
                    add_dep_helper(dvb.ins, prev_prod, True, "DVE WAW")
                    blk_pre = [aab1.ins, aab2.ins, dvb.ins]
                for p in range(NPAIR):
                    it = b * NPAIR + p
                    ps_s = ps.tile([128, BLK], mybir.dt.float32, tag="ps_s")
                    mm1 = nc.tensor.matmul(
                        ps_s[:], mm1w[:, 128 * p:128 * (p + 1)],
                        xaug[:, BLK * b:BLK * (b + 1)], start=True, stop=True)
                    kfu = kp.tile([128, BLK], FR, tag="kfu")
                    ex = nc.scalar.activation(
                        kfu[:], ps_s[:], mybir.ActivationFunctionType.Exp)
                    for pre in blk_pre:
                        add_dep_helper(ex.ins, pre, False, "after blk absorb")
                    exp_hist.append(ex.ins)
                    last_act = ex.ins
                    # absorb the ps_t WAR (DVE prod 3 iterations back) so the
                    # 1-wait-slot f32r mm2 only waits on ACT(exp)
                    if it >= 3:
                        ldw = nc.tensor.ldweights(dummy_bf[:])
                        add_dep_helper(ldw.ins, prod_hist[it - 3], True,
                                       "absorb ps_t WAR")
                    ps_t = ps.tile([128, BLK], mybir.dt.float32, tag="ps_t")
                    mm2 = nc.tensor.matmul(ps_t[:], cr[:, 128 * p:128 * (p + 1)],
                                           kfu[:], start=True, stop=True)
                    # absorb exp's ACT sem on DVE so the 1-slot TT only
                    # waits on PE (mm2). Unique dummy tile per iteration —
                    # reusing one adds a same-engine WAW sem wait.
                    ddv = sb.tile([1, 1], mybir.dt.float32, tag=f"ddv{it}")
                    dab = nc.vector.memset(ddv[:], 0.0)
                    add_dep_helper(dab.ins, ex.ins, True, "absorb exp for DVE")
                    g = gp.tile([128, BLK], FR, tag="g")
                    pr = nc.vector.tensor_tensor(g[:], kfu[:], ps_t[:],
                                                 mybir.AluOpType.mult)
                    add_dep_helper(pr.ins, dab.ins, False, "order after absorb")
                    prod_hist.append(pr.ins)
                    last_dve = pr.ins
                    mm3a = nc.tensor.matmul(
                        ps_o[:], cr[:, 1024 + 32 * p:1024 + 32 * (p + 1)],
                        g[:], start=False, stop=False)
                    mm3b = nc.tensor.matmul(
                        ps_o[:], cr[:, 1280 + 32 * p:1280 + 32 * (p + 1)],
                        kfu[:], start=False, stop=(p == NPAIR - 1))
                    last_pe = mm3b.ins
                sc = nc.scalar.copy(staging[:, BLK * b:BLK * (b + 1)], ps_o[:])
                last_act = sc.ins

            out_dma = nc.sync.dma_start(out=out_ext[:], in_=staging[:]).ins
            funnel.append(out_dma)
            funnel += [last_act, last_pe, last_dve]
            for dep in funnel:
                nop = nc.sync.nop(nofuse=True)
                add_dep_helper(nop.ins, dep, True, "tail funnel")
    return nc


def kernel(x, z, u_mean, u_tril_vec, log_ls, log_var):
    from concourse.bass_utils import run_bass_kernel_spmd

    if "nc" not in _cache:
        _cache["nc"] = _build_program()
    nc = _cache["nc"]

    xaug, mm1w, cR, mmvw = _host_precompute(
        np.asarray(x), np.asarray(z), np.asarray(u_mean),
        np.asarray(u_tril_vec), np.asarray(log_ls), np.asarray(log_var))

    in_maps = []
    for c in range(NCORES):
        in_maps.append({
            "xaug": np.ascontiguousarray(xaug[:, c * N_LOC:(c + 1) * N_LOC]),
            "mm1w": mm1w,
            "cR": cR.view(np.float32),
            "mmvw": mmvw,
        })
    res = run_bass_kernel_spmd(nc, in_maps, list(range(NCORES)))
    outs = [res.results[c]["outp"] for c in range(NCORES)]
    full = np.concatenate(outs, axis=1)          # [32, N]
    pred_var = full[0:NHO].reshape(H, O, N).astype(np.float32)
    pred_mu = full[NHO:2 * NHO].reshape(H, O, N).astype(np.float32)
    return pred_mu, pred_var
